# revision 20
# baseline (speedup 1.0000x reference)
"""Trainium2 Bass kernel for nn_CDFLoss (masked-BCE CDF loss + monotonicity penalty).

Reference semantics (see harness reference):
    target[i,t] = (event[i]==1) & (t >= duration[i])
    mask[i,t]   = (event[i]==1) | (t <= duration[i])
    p  = clip(F_pred, EPS, 1-EPS)
    bce = sum(mask * -(target*log(p) + (1-target)*log1p(-p))) / sum(mask)
    mono = mean(relu(F_pred[:,:-1] - F_pred[:,1:] + 0.1))
    loss = bce + 0.1*mono + 0.1*mean(biases**2)

Device strategy (data-parallel over 8 cores, 4096 rows each, fp16 on-chip).
Per element, with per-row threshold thr = dur + 0.5 - ev, c2 = ev/2,
s2 = -(1+ev), B1 = 0.5 + 1e-7, the BCE log argument is
    q = (x - 0.5) * g * s2 + B1,   g = [t < thr] - c2
which evaluates to 1-x / x / B1 for the target=0 / target=1 / masked-out
regions respectively.  Per [128,1024] tile:
    DVE : g   = (iota < thr) - c2                   (dual-op tensor_scalar)
          q0  = (x - 0.5) * g                       (scalar_tensor_tensor)
          mono= sum_t max(x_t + 0.1, x_{t+1})       (stt with accum_out)
    ACT : Ln(q0 * s2 + B1) with accum_out -> row sums of ln q
    PE  : ones^T @ x accumulated in PSUM -> column sums of x (for the mono
          telescope correction sum_t x_{t+1} = rowsum - x_0)
    DMA : SWDGE f32->fp16 cast loads (separate tile tags so they overlap);
          HWDGE SBUF->SBUF shifted copy xs[c] = x[c+1]
Host combines per-core partial sums, removes the masked-out Ln(B1) constant
via an on-device probe, and fixes up the ~1e-4 fraction of elements whose
f32 value rounds to fp16 1.0 (their 1-x is unrepresentable on device).
"""

import numpy as np

import concourse.bacc as bacc
import concourse.mybir as mybir
from concourse import tile
from concourse.bass_utils import run_bass_kernel_spmd

F32 = mybir.dt.float32
F16 = mybir.dt.float16
I32 = mybir.dt.int32
OP = mybir.AluOpType
AF = mybir.ActivationFunctionType

B_FULL = 32768
T = 1024
N_CORES = 8
ROWS = B_FULL // N_CORES          # rows per core
MONO_MARGIN = 0.1
MONO_W = 0.1
BETA = 0.1
EPS = 1e-7
# bias fed to ACT: q0*s2 + B1.  float32(0.5 + 1e-7)
B1 = float(np.float32(np.float64(0.5) + np.float64(1e-7)))
# f32 values >= this round to 1.0 in fp16 (RN ties-even); 1 - 2^-12
FP16_ONE_THR = np.float32(1.0 - 2.0**-12)

_CACHE = {}


def build_module(rows=ROWS, num_devices=N_CORES, repeat=1, dma_tags=4,
                 xs_mode="shift", q0_dtype=F16, bufs=2):
    """Build + compile the per-core Bass module (SPMD: same program/core data)."""
    assert rows % 128 == 0
    tiles = rows // 128

    nc = bacc.Bacc(
        "TRN2",
        debug=False,
        enable_asserts=False,
        target_bir_lowering=False,
        num_devices=num_devices,
    )

    f_in = nc.dram_tensor("F", [rows, T], F32, kind="ExternalInput")
    thr_in = nc.dram_tensor("thr", [rows], F32, kind="ExternalInput")
    c2_in = nc.dram_tensor("c2", [rows], F32, kind="ExternalInput")
    s2_in = nc.dram_tensor("s2", [rows], F32, kind="ExternalInput")

    ln_out = nc.dram_tensor("lnacc", [128, tiles], F32, kind="ExternalOutput")
    m_out = nc.dram_tensor("mono", [128, tiles], F32, kind="ExternalOutput")
    x_out = nc.dram_tensor("xsum", [1, T], F32, kind="ExternalOutput")
    p_out = nc.dram_tensor("probe", [1, 2], F32, kind="ExternalOutput")

    f_ap = f_in.ap()

    with tile.TileContext(nc) as tc:
        with (
            tc.tile_pool(name="const", bufs=1) as cpool,
            tc.tile_pool(name="x", bufs=bufs) as xpool,
            tc.tile_pool(name="work", bufs=bufs) as wpool,
            tc.tile_pool(name="psum", bufs=1, space="PSUM") as ppool,
        ):
            # --- one-time setup ---
            iota32 = cpool.tile([128, T], I32)
            nc.gpsimd.iota(iota32[:, :], pattern=[[1, T]], base=0,
                           channel_multiplier=0)
            iota16 = cpool.tile([128, T], F16)
            nc.vector.tensor_scalar_add(iota16[:, :], iota32[:, :], 0.0)

            thr_sb = cpool.tile([128, tiles], F32)
            c2_sb = cpool.tile([128, tiles], F32)
            s2_sb = cpool.tile([128, tiles], F32)
            nc.sync.dma_start(thr_sb[:, :],
                              thr_in.ap().rearrange("(k p) -> p k", p=128))
            nc.sync.dma_start(c2_sb[:, :],
                              c2_in.ap().rearrange("(k p) -> p k", p=128))
            nc.sync.dma_start(s2_sb[:, :],
                              s2_in.ap().rearrange("(k p) -> p k", p=128))

            ln_sb = cpool.tile([128, tiles], F32)
            m_sb = cpool.tile([128, tiles], F32)
            nc.vector.memset(ln_sb[:, :], 0.0)
            nc.vector.memset(m_sb[:, :], 0.0)

            b1_sb = cpool.tile([128, 1], F32)
            nc.vector.memset(b1_sb[:, :], B1)
            ones_sb = cpool.tile([128, 1], F16)
            nc.vector.memset(ones_sb[:, :], 1.0)

            # probes: Ln(0*1 + B1) and Ln(-0.5*1 + B1)
            pconst = cpool.tile([1, 2], F32)
            nc.vector.memset(pconst[:, 0:1], 0.0)
            nc.vector.memset(pconst[:, 1:2], -0.5)
            probe_sb = cpool.tile([1, 2], F32)
            nc.scalar.activation(probe_sb[:, :], pconst[:, :], AF.Ln,
                                 bias=b1_sb[0:1, :], scale=1.0)
            nc.sync.dma_start(p_out.ap(), probe_sb[:, :])

            # PSUM accumulators for column sums of x (two N=512 halves)
            ps0 = ppool.tile([1, 512], F32)
            ps1 = ppool.tile([1, 512], F32)

            n_mm = repeat * tiles

            # --- main loop over tiles ---
            mm = 0
            for k_ in [k for _ in range(repeat) for k in range(tiles)]:
                k = k_
                x = xpool.tile([128, T], F16, tag=f"x{k % dma_tags}")
                nc.gpsimd.dma_start(x[:, :], f_ap[k * 128:(k + 1) * 128, :])

                # BCE selector and log argument
                g_t = wpool.tile([128, T], F16, tag="g")
                nc.vector.tensor_scalar(
                    out=g_t[:, :], in0=iota16[:, :],
                    scalar1=thr_sb[:, k:k + 1], scalar2=c2_sb[:, k:k + 1],
                    op0=OP.is_lt, op1=OP.subtract,
                )
                q0 = wpool.tile([128, T], q0_dtype, tag="q0")
                nc.vector.scalar_tensor_tensor(
                    out=q0[:, :], in0=x[:, :], scalar=0.5,
                    in1=g_t[:, :], op0=OP.subtract, op1=OP.mult,
                )
                lnscr = wpool.tile([128, T], F16, tag="ln")
                nc.scalar.activation(
                    lnscr[:, :], q0[:, :], AF.Ln,
                    bias=b1_sb[:, :], scale=s2_sb[:, k:k + 1],
                    accum_out=ln_sb[:, k:k + 1],
                )

                # mono: sum_t max(x_t + 0.1, x_{t+1})
                if xs_mode == "dma":
                    xs = wpool.tile([128, T], F16, tag=f"xs{k % 2}")
                    nc.sync.dma_start(xs[:, 0:T - 1], x[:, 1:T])
                    xs_view = xs[:, 0:T - 1]
                else:  # "shift": read x shifted directly (unaligned, 1x mode)
                    xs_view = x[:, 1:T]
                mscr = wpool.tile([128, T], F16, tag="m")
                nc.vector.scalar_tensor_tensor(
                    out=mscr[:, 0:T - 1], in0=x[:, 0:T - 1], scalar=MONO_MARGIN,
                    in1=xs_view, op0=OP.add, op1=OP.max,
                    accum_out=m_sb[:, k:k + 1],
                )

                # column sums of x via TensorE (for mono telescope correction)
                nc.tensor.matmul(ps0[:, :], ones_sb[:, :], x[:, 0:512],
                                 start=(mm == 0), stop=(mm == n_mm - 1))
                nc.tensor.matmul(ps1[:, :], ones_sb[:, :], x[:, 512:T],
                                 start=(mm == 0), stop=(mm == n_mm - 1))
                mm += 1

            xsum_sb = cpool.tile([1, T], F32)
            nc.vector.tensor_scalar_add(xsum_sb[:, 0:512], ps0[:, :], 0.0)
            nc.vector.tensor_scalar_add(xsum_sb[:, 512:T], ps1[:, :], 0.0)

            nc.sync.dma_start(ln_out.ap(), ln_sb[:, :])
            nc.sync.dma_start(m_out.ap(), m_sb[:, :])
            nc.sync.dma_start(x_out.ap(), xsum_sb[:, :])

    nc.compile()
    return nc


def _get_module(rows=ROWS, num_devices=N_CORES):
    key = (rows, num_devices)
    if key not in _CACHE:
        _CACHE[key] = build_module(rows, num_devices)
    return _CACHE[key]


def make_in_maps(F_pred, duration, event, n_cores=N_CORES, rows=ROWS):
    """Per-core input dicts. F slices are zero-copy contiguous views."""
    F_pred = np.asarray(F_pred, dtype=np.float32)
    dur = np.asarray(duration).astype(np.float32)
    ev = np.asarray(event).astype(np.float32)
    thr = (dur + np.float32(0.5) - ev).astype(np.float32)
    c2 = (ev * np.float32(0.5)).astype(np.float32)
    s2 = (-(1.0 + ev)).astype(np.float32)
    in_maps = []
    for c in range(n_cores):
        sl = slice(c * rows, (c + 1) * rows)
        in_maps.append({
            "F": F_pred[sl],
            "thr": np.ascontiguousarray(thr[sl]),
            "c2": np.ascontiguousarray(c2[sl]),
            "s2": np.ascontiguousarray(s2[sl]),
        })
    return in_maps


def combine(results, F_pred, biases, duration, event, n_cores=N_CORES, rows=ROWS):
    """Host-side reduction of per-core partial sums into the final scalar loss."""
    F_pred = np.asarray(F_pred, dtype=np.float32)
    dur = np.asarray(duration).astype(np.int64)
    ev = np.asarray(event).astype(np.int64)
    B = n_cores * rows

    P1 = np.float64(results[0]["probe"][0, 0])  # ACT Ln(B1)
    P2 = np.float64(results[0]["probe"][0, 1])  # ACT Ln(B1 - 0.5)

    ln_total = np.float64(0.0)
    mono_total = np.float64(0.0)
    mask_total = np.float64(0.0)

    for c in range(n_cores):
        sl = slice(c * rows, (c + 1) * rows)
        r = results[c]
        d = dur[sl]
        e = ev[sl]

        ln_sum = np.float64(r["lnacc"].astype(np.float64).sum())
        m_sum = np.float64(r["mono"].astype(np.float64).sum())
        x_sum = np.float64(r["xsum"].astype(np.float64).sum())

        # remove masked-out constant contributions: ev=0 rows, t>dur -> Ln(B1)
        count0 = np.where(e == 0, (T - 1) - d, 0).sum()
        ln_sum -= np.float64(count0) * P1

        # fp16-saturation fixup: f32 x >= FP16_ONE_THR became exactly 1.0 on
        # device; in the (t < thr) branch the device computed Ln(B1-0.5).
        Fc = F_pred[sl]
        ii, tt = np.nonzero(Fc >= FP16_ONE_THR)
        if ii.size:
            thr_rows = d[ii] + 0.5 - e[ii]
            uu = tt < thr_rows
            if uu.any():
                x = Fc[ii[uu], tt[uu]].astype(np.float64)
                true_ln = np.log1p(-np.minimum(x, np.float64(np.float32(1.0 - EPS))))
                ln_sum += (true_ln - P2).sum()

        ln_total += ln_sum
        mask_total += np.where(e == 1, T, d + 1).sum()

        # mono: m_sum counted sum_t max(x_t+0.1, x_{t+1}) over t in [0,1022]
        #     = relu_sum + sum_rows (rowsum16(x) - x0_16)
        x0_16 = Fc[:, 0].astype(np.float16).astype(np.float64).sum()
        mono_total += m_sum - x_sum + x0_16

    bce = -ln_total / mask_total
    mono_mean = mono_total / (np.float64(B) * (T - 1))
    bias_term = np.float64(BETA) * np.mean(np.asarray(biases, np.float64) ** 2)
    loss = bce + np.float64(MONO_W) * mono_mean + bias_term
    return np.float32(loss)


def run(F_pred, biases, duration, event, **spmd_kwargs):
    nc = _get_module()
    in_maps = make_in_maps(F_pred, duration, event)
    res = run_bass_kernel_spmd(nc, in_maps, core_ids=list(range(N_CORES)),
                               **spmd_kwargs)
    return combine(res.results, F_pred, biases, duration, event), res


def kernel(F_pred, biases, duration, event):
    F_pred = np.asarray(F_pred)
    assert F_pred.shape == (B_FULL, T), f"unexpected shape {F_pred.shape}"
    return run(F_pred, biases, duration, event)[0]
